# revision 10
# baseline (speedup 1.0000x reference)
"""Trainium2 Bass kernel for nn_AtomicLinear.

Math: reference computes (x[:, None, :] * weight)[:, :, indices].sum(2) + bias,
which equals sum_i x[b, idx[i]] * w[o, idx[i]] = sum_j c_j * x[b, j] * w[o, j]
with c_j = multiplicity of j in indices.  So the whole op is a plain GEMM
against a count-scaled weight:  out = x @ (weight * c).T + bias.

Device strategy: shard batch into BG groups x out_features into OG groups
(BG*OG = 8 cores).  Each core computes out_blk.T = W_t.T @ x_blk.T with
W_t = (weight * c).T [in, out/OG] (host pre-transposed), x_blk.T [in, B/BG]
(host pre-transposed).  All DMAs are large contiguous-chunk slab loads:
  lhsT = W_t tile  [K=128 in, M=128 out]   (stationary)
  rhs  = x.T tile  [K=128 in, N=512 batch] (moving)
  psum [M=128 out, N=512 batch], accumulated over 16 K-tiles.
Bias is folded in during the PSUM->SBUF eviction on the scalar engine.
"""

import numpy as np

B = 4096
IN_F = 2048
OUT_F = 2048
N_CORES = 8

KT = IN_F // 128  # 16 contraction tiles

TRACE = False
LAST_EXEC_TIME_NS = None
LAST_RESULTS = None

PE_DTYPE = "float16"  # 1 cycle/row on PE; rel err ~2.9e-4 (gate is 2e-2)
BG = 4  # batch groups
OG = 2  # out-feature groups

_prog_cache = {}


def _shapes(bg, og):
    b_sh = B // bg
    out_sh = OUT_F // og
    mt = out_sh // 128
    nt = b_sh // 512
    return b_sh, out_sh, mt, nt


def _build_program(pe_dtype_name=None, bg=None, og=None, repeats=1, w_bufs=4,
                   xch=4, x_eng="gpsimd", ps_bufs=4):
    import concourse.tile as tile
    from concourse import bacc, mybir

    pe_dtype_name = pe_dtype_name or PE_DTYPE
    bg = bg or BG
    og = og or OG

    key = (pe_dtype_name, bg, og, repeats, w_bufs, xch, x_eng, ps_bufs)
    if key in _prog_cache:
        return _prog_cache[key]

    b_sh, out_sh, mt, nt = _shapes(bg, og)
    pe_dt = getattr(mybir.dt, pe_dtype_name)
    f32 = mybir.dt.float32

    nc = bacc.Bacc(
        "TRN2", target_bir_lowering=False, debug=False, num_devices=N_CORES
    )

    xt_d = nc.dram_tensor("xt", [IN_F, b_sh], pe_dt, kind="ExternalInput").ap()
    wt_d = nc.dram_tensor("wt", [IN_F, out_sh], pe_dt, kind="ExternalInput").ap()
    bias_d = nc.dram_tensor("biaspm", [128, mt], f32, kind="ExternalInput").ap()
    out_d = nc.dram_tensor("outT", [out_sh, b_sh], f32, kind="ExternalOutput").ap()

    XCH = xch  # x loaded in XCH chunks so PE can start before the full load

    with tile.TileContext(nc) as tc:
        with (
            tc.tile_pool(name="xsb", bufs=2) as xpool,
            tc.tile_pool(name="wsb", bufs=w_bufs) as wpool,
            tc.tile_pool(name="bsb", bufs=1) as bpool,
            tc.tile_pool(name="osb", bufs=4) as opool,
            tc.tile_pool(name="ps", bufs=ps_bufs, space="PSUM") as pspool,
        ):
            bias_sb = bpool.tile([128, mt], f32)
            nc.gpsimd.dma_start(out=bias_sb[:], in_=bias_d[:])

            # dram views with 128-partition tiling folded out
            xt_v = xt_d.rearrange("(t p) f -> p t f", p=128)  # [128, KT, b_sh]
            wt_v = wt_d.rearrange("(t p) f -> p t f", p=128)  # [128, KT, out_sh]

            # repeats > 1 re-runs the identical computation inside one NEFF;
            # used for benchmarking (HW time = delta between repeat counts).
            for _rep in range(repeats):
                x_sb = xpool.tile([128, KT * b_sh], pe_dt)
                x_v = x_sb[:].rearrange("p (t f) -> p t f", t=KT)
                kc = KT // XCH
                x_engine = getattr(nc, x_eng)
                for i in range(XCH):
                    x_engine.dma_start(
                        out=x_v[:, i * kc : (i + 1) * kc, :],
                        in_=xt_v[:, i * kc : (i + 1) * kc, :],
                    )

                for m in range(mt):
                    w_m = wpool.tile([128, KT * 128], pe_dt)
                    w_v = w_m[:].rearrange("p (t f) -> p t f", t=KT)
                    nc.sync.dma_start(
                        out=w_v[:],
                        in_=wt_v[:, :, m * 128 : (m + 1) * 128],
                    )
                    for n in range(nt):
                        ps = pspool.tile([128, 512], f32)
                        for k in range(KT):
                            nc.tensor.matmul(
                                ps[:],
                                lhsT=w_m[:, k * 128 : (k + 1) * 128],
                                rhs=x_sb[
                                    :, k * b_sh + n * 512 : k * b_sh + (n + 1) * 512
                                ],
                                start=(k == 0),
                                stop=(k == KT - 1),
                            )
                        ot = opool.tile([128, 512], f32)
                        nc.scalar.activation(
                            ot[:],
                            ps[:],
                            mybir.ActivationFunctionType.Identity,
                            bias=bias_sb[:, m : m + 1],
                        )
                        nc.scalar.dma_start(
                            out=out_d[
                                m * 128 : (m + 1) * 128, n * 512 : (n + 1) * 512
                            ],
                            in_=ot[:],
                        )

    nc.compile()
    _prog_cache[key] = nc
    return nc


def _prep_host(x, weight, bias, indices, pe_dtype_name=None, bg=None, og=None):
    from concourse import mybir

    pe_dtype_name = pe_dtype_name or PE_DTYPE
    bg = bg or BG
    og = og or OG
    b_sh, out_sh, mt, nt = _shapes(bg, og)
    np_dt = mybir.dt.np(getattr(mybir.dt, pe_dtype_name))

    x = np.asarray(x, dtype=np.float32)
    weight = np.asarray(weight, dtype=np.float32)
    bias = np.asarray(bias, dtype=np.float32)
    idx = np.asarray(indices).astype(np.int64)

    counts = np.bincount(idx, minlength=IN_F).astype(np.float32)
    w_t = np.ascontiguousarray((weight * counts[None, :]).T).astype(np_dt)  # [in, out]
    xt_full = np.ascontiguousarray(x.T).astype(np_dt)  # [in, B]

    in_maps = []
    for c in range(N_CORES):
        bgi, ogi = divmod(c, og)
        bias_blk = bias[ogi * out_sh : (ogi + 1) * out_sh]
        in_maps.append(
            {
                "xt": np.ascontiguousarray(
                    xt_full[:, bgi * b_sh : (bgi + 1) * b_sh]
                ),
                "wt": np.ascontiguousarray(
                    w_t[:, ogi * out_sh : (ogi + 1) * out_sh]
                ),
                "biaspm": np.ascontiguousarray(bias_blk.reshape(mt, 128).T),
            }
        )
    return in_maps


def _gather_out(results, bg=None, og=None):
    bg = bg or BG
    og = og or OG
    b_sh, out_sh, mt, nt = _shapes(bg, og)
    out = np.empty((B, OUT_F), dtype=np.float32)
    for c in range(N_CORES):
        bgi, ogi = divmod(c, og)
        out[
            bgi * b_sh : (bgi + 1) * b_sh, ogi * out_sh : (ogi + 1) * out_sh
        ] = results[c]["outT"].T
    return out


def kernel(x, weight, bias, indices):
    global LAST_EXEC_TIME_NS, LAST_RESULTS
    from concourse.bass_utils import run_bass_kernel_spmd

    in_maps = _prep_host(x, weight, bias, indices)
    nc = _build_program()

    res = run_bass_kernel_spmd(nc, in_maps, list(range(N_CORES)), trace=TRACE)
    LAST_EXEC_TIME_NS = res.exec_time_ns
    LAST_RESULTS = res
    return _gather_out(res.results)


# revision 13
# speedup vs baseline: 1.8685x; 1.8685x over previous
"""Trainium2 Bass kernel for nn_AtomicLinear.

Math: reference computes (x[:, None, :] * weight)[:, :, indices].sum(2) + bias,
which equals sum_i x[b, idx[i]] * w[o, idx[i]] = sum_j c_j * x[b, j] * w[o, j]
with c_j = multiplicity of j in indices.  So the whole op is a plain GEMM
against a count-scaled weight:  out = x @ (weight * c).T + bias.

Device strategy: shard batch into BG groups x out_features into OG groups
(BG*OG = 8 cores).  Each core computes out_blk.T = W_t.T @ x_blk.T with
W_t = (weight * c).T [in, out/OG] (host pre-transposed), x_blk.T [in, B/BG]
(host pre-transposed).  All DMAs are large contiguous-chunk slab loads:
  lhsT = W_t tile  [K=128 in, M=128 out]   (stationary)
  rhs  = x.T tile  [K=128 in, N=512 batch] (moving)
  psum [M=128 out, N=512 batch], accumulated over 16 K-tiles.
Bias is folded in during the PSUM->SBUF eviction on the scalar engine.
"""

import numpy as np

B = 4096
IN_F = 2048
OUT_F = 2048
N_CORES = 8

KT = IN_F // 128  # 16 contraction tiles

TRACE = False
LAST_EXEC_TIME_NS = None
LAST_RESULTS = None

PE_DTYPE = "float16"  # 1 cycle/row on PE; rel err ~2.9e-4 (gate is 2e-2)
BG = 4  # batch groups
OG = 2  # out-feature groups

_prog_cache = {}


def _shapes(bg, og):
    b_sh = B // bg
    out_sh = OUT_F // og
    mt = out_sh // 128
    nt = b_sh // 512
    return b_sh, out_sh, mt, nt


def _build_program(pe_dtype_name=None, bg=None, og=None, repeats=1, w_bufs=4,
                   xch=4, x_eng="gpsimd", ps_bufs=4, evict="act",
                   out_eng="scalar"):
    import concourse.tile as tile
    from concourse import bacc, mybir

    pe_dtype_name = pe_dtype_name or PE_DTYPE
    bg = bg or BG
    og = og or OG

    key = (pe_dtype_name, bg, og, repeats, w_bufs, xch, x_eng, ps_bufs,
           evict, out_eng)
    if key in _prog_cache:
        return _prog_cache[key]

    b_sh, out_sh, mt, nt = _shapes(bg, og)
    pe_dt = getattr(mybir.dt, pe_dtype_name)
    f32 = mybir.dt.float32

    nc = bacc.Bacc(
        "TRN2", target_bir_lowering=False, debug=False, num_devices=N_CORES
    )

    xt_d = nc.dram_tensor("xt", [IN_F, b_sh], pe_dt, kind="ExternalInput").ap()
    wt_d = nc.dram_tensor("wt", [IN_F, out_sh], pe_dt, kind="ExternalInput").ap()
    bias_d = nc.dram_tensor("biaspm", [128, mt], f32, kind="ExternalInput").ap()
    out_d = nc.dram_tensor("outT", [out_sh, b_sh], f32, kind="ExternalOutput").ap()

    XCH = xch  # x loaded in XCH chunks so PE can start before the full load

    with tile.TileContext(nc) as tc:
        with (
            tc.tile_pool(name="xsb", bufs=2) as xpool,
            tc.tile_pool(name="wsb", bufs=w_bufs) as wpool,
            tc.tile_pool(name="bsb", bufs=1) as bpool,
            tc.tile_pool(name="osb", bufs=4) as opool,
            tc.tile_pool(name="ps", bufs=ps_bufs, space="PSUM") as pspool,
        ):
            bias_sb = bpool.tile([128, mt], f32)
            nc.gpsimd.dma_start(out=bias_sb[:], in_=bias_d[:])

            # dram views with 128-partition tiling folded out
            xt_v = xt_d.rearrange("(t p) f -> p t f", p=128)  # [128, KT, b_sh]
            wt_v = wt_d.rearrange("(t p) f -> p t f", p=128)  # [128, KT, out_sh]

            # repeats > 1 re-runs the identical computation inside one NEFF;
            # used for benchmarking (HW time = delta between repeat counts).
            for _rep in range(repeats):
                x_sb = xpool.tile([128, KT * b_sh], pe_dt)
                x_v = x_sb[:].rearrange("p (t f) -> p t f", t=KT)
                kc = KT // XCH
                x_engine = getattr(nc, x_eng)
                for i in range(XCH):
                    x_engine.dma_start(
                        out=x_v[:, i * kc : (i + 1) * kc, :],
                        in_=xt_v[:, i * kc : (i + 1) * kc, :],
                    )

                for m in range(mt):
                    w_m = wpool.tile([128, KT * 128], pe_dt)
                    w_v = w_m[:].rearrange("p (t f) -> p t f", t=KT)
                    nc.sync.dma_start(
                        out=w_v[:],
                        in_=wt_v[:, :, m * 128 : (m + 1) * 128],
                    )
                    for n in range(nt):
                        ps = pspool.tile([128, 512], f32)
                        for k in range(KT):
                            nc.tensor.matmul(
                                ps[:],
                                lhsT=w_m[:, k * 128 : (k + 1) * 128],
                                rhs=x_sb[
                                    :, k * b_sh + n * 512 : k * b_sh + (n + 1) * 512
                                ],
                                start=(k == 0),
                                stop=(k == KT - 1),
                            )
                        ot = opool.tile([128, 512], f32)
                        use_dve = evict == "dve" or (
                            evict == "split" and (m * nt + n) % 2 == 1
                        )
                        if use_dve:
                            nc.vector.tensor_scalar_add(
                                ot[:], ps[:], bias_sb[:, m : m + 1]
                            )
                        else:
                            nc.scalar.activation(
                                ot[:],
                                ps[:],
                                mybir.ActivationFunctionType.Identity,
                                bias=bias_sb[:, m : m + 1],
                            )
                        getattr(nc, out_eng).dma_start(
                            out=out_d[
                                m * 128 : (m + 1) * 128, n * 512 : (n + 1) * 512
                            ],
                            in_=ot[:],
                        )

    nc.compile()
    _prog_cache[key] = nc
    return nc


def _prep_host(x, weight, bias, indices, pe_dtype_name=None, bg=None, og=None):
    from concourse import mybir

    pe_dtype_name = pe_dtype_name or PE_DTYPE
    bg = bg or BG
    og = og or OG
    b_sh, out_sh, mt, nt = _shapes(bg, og)
    np_dt = mybir.dt.np(getattr(mybir.dt, pe_dtype_name))

    x = np.asarray(x, dtype=np.float32)
    weight = np.asarray(weight, dtype=np.float32)
    bias = np.asarray(bias, dtype=np.float32)
    idx = np.asarray(indices).astype(np.int64)

    counts = np.bincount(idx, minlength=IN_F).astype(np.float32)
    w_t = np.ascontiguousarray((weight * counts[None, :]).T).astype(np_dt)  # [in, out]
    xt_full = np.ascontiguousarray(x.T).astype(np_dt)  # [in, B]

    in_maps = []
    for c in range(N_CORES):
        bgi, ogi = divmod(c, og)
        bias_blk = bias[ogi * out_sh : (ogi + 1) * out_sh]
        in_maps.append(
            {
                "xt": np.ascontiguousarray(
                    xt_full[:, bgi * b_sh : (bgi + 1) * b_sh]
                ),
                "wt": np.ascontiguousarray(
                    w_t[:, ogi * out_sh : (ogi + 1) * out_sh]
                ),
                "biaspm": np.ascontiguousarray(bias_blk.reshape(mt, 128).T),
            }
        )
    return in_maps


def _gather_out(results, bg=None, og=None):
    bg = bg or BG
    og = og or OG
    b_sh, out_sh, mt, nt = _shapes(bg, og)
    out = np.empty((B, OUT_F), dtype=np.float32)
    for c in range(N_CORES):
        bgi, ogi = divmod(c, og)
        out[
            bgi * b_sh : (bgi + 1) * b_sh, ogi * out_sh : (ogi + 1) * out_sh
        ] = results[c]["outT"].T
    return out


_runner_cache = {}


def _get_runner(nc):
    """Cached jitted executor for the compiled bass program (the
    run_bass_kernel_spmd axon path rebuilds its jit closure per call;
    caching avoids re-tracing on repeat kernel() invocations)."""
    if id(nc) in _runner_cache:
        return _runner_cache[id(nc)]

    import jax
    from jax.sharding import Mesh, PartitionSpec
    from jax.experimental.shard_map import shard_map
    import concourse.mybir as mybir
    from concourse.bass2jax import (
        _bass_exec_p,
        install_neuronx_cc_hook,
        partition_id_tensor,
    )

    install_neuronx_cc_hook()
    partition_name = nc.partition_id_tensor.name if nc.partition_id_tensor else None
    in_names, out_names, out_avals, zero_shapes = [], [], [], []
    for alloc in nc.m.functions[0].allocations:
        if not isinstance(alloc, mybir.MemoryLocationSet):
            continue
        name = alloc.memorylocations[0].name
        if alloc.kind == "ExternalInput":
            if name != partition_name:
                in_names.append(name)
        elif alloc.kind == "ExternalOutput":
            out_names.append(name)
            shape = tuple(alloc.tensor_shape)
            dtype = mybir.dt.np(alloc.dtype)
            out_avals.append(jax.core.ShapedArray(shape, dtype))
            zero_shapes.append((shape, dtype))
    all_in_names = list(in_names) + list(out_names)
    if partition_name is not None:
        all_in_names.append(partition_name)

    def _body(*args):
        operands = list(args)
        if partition_name is not None:
            operands.append(partition_id_tensor())
        return tuple(
            _bass_exec_p.bind(
                *operands,
                out_avals=tuple(out_avals),
                in_names=tuple(all_in_names),
                out_names=tuple(out_names),
                lowering_input_output_aliases=(),
                sim_require_finite=True,
                sim_require_nnan=True,
                nc=nc,
            )
        )

    devices = jax.devices()[:N_CORES]
    mesh = Mesh(np.asarray(devices), ("core",))
    n_io = len(in_names) + len(out_names)
    fn = jax.jit(
        shard_map(
            _body,
            mesh=mesh,
            in_specs=(PartitionSpec("core"),) * n_io,
            out_specs=(PartitionSpec("core"),) * len(out_names),
            check_rep=False,
        ),
        keep_unused=True,
    )

    runner = (fn, in_names, out_names, out_avals, zero_shapes)
    _runner_cache[id(nc)] = runner
    return runner


def kernel(x, weight, bias, indices):
    global LAST_RESULTS
    import jax

    in_maps = _prep_host(x, weight, bias, indices)
    nc = _build_program()
    fn, in_names, out_names, out_avals, zero_shapes = _get_runner(nc)

    concat_in = [
        np.concatenate([np.asarray(m[name]) for m in in_maps], axis=0)
        for name in in_names
    ]
    concat_zeros = [
        np.zeros((N_CORES * s[0], *s[1:]), dt) for s, dt in zero_shapes
    ]
    outs = fn(*concat_in, *concat_zeros)
    results = [
        {
            name: np.asarray(outs[i]).reshape(N_CORES, *out_avals[i].shape)[c]
            for i, name in enumerate(out_names)
        }
        for c in range(N_CORES)
    ]
    LAST_RESULTS = results
    return _gather_out(results)
